# revision 39
# baseline (speedup 1.0000x reference)
"""nn_CrossAttention Trainium2 kernel — 8-core data-parallel over batch.

Per core (batch slice b=1):
  q1^T / kv1^T via transposed-orientation 1x1 convs (stationary = input rows,
  f32r matmuls), depthwise 3x3/7x7 as per-(channel, dh) banded-Toeplitz
  matmuls on the TensorEngine (host-built bf16 Toeplitz tiles, PSUM
  accumulation over dh with free-dim h shifts), l2-norm folded into attention
  scales, QK^T with n on partitions, softmax without max-subtraction
  (|logits| <= temperature), attn@v merged with the output 1x1 conv through
  a per-pair [96,192] fused matrix.
"""

import sys

sys.path.insert(0, "/opt/trn_rl_repo")

import numpy as np
import ml_dtypes

B, C, Himg, Wimg = 8, 192, 128, 128
HW = Himg * Wimg
HEADS, DHC = 4, 48      # heads, channels per head
PC = 96                 # channels per head-pair
NW = 8                  # Toeplitz tiles per DMA wave
SLAB = 4                # image rows per input stream slab

_PROG = None            # cached (nc, meta)


def _build_toeplitz(wdw, ksz, dtype):
    """wdw [c, ksz, ksz] f32 -> [128, c*ksz, 128], tile index = c*ksz + dh.

    T[w_in, tile, w_out] = wdw[c, dh, w_in - w_out + pad] inside the band,
    else 0.  Partition-major so a wave DMA reads contiguous bytes per
    partition.
    """
    pad = ksz // 2
    wi = np.arange(128)[:, None]
    wo = np.arange(128)[None, :]
    idx = wi - wo + pad
    valid = (idx >= 0) & (idx < ksz)
    idxc = np.clip(idx, 0, ksz - 1)
    T = wdw[:, :, idxc] * valid[None, None]          # [c, ksz, 128, 128]
    T = T.reshape(-1, 128, 128).transpose(1, 0, 2)   # [128, c*ksz, 128]
    return np.ascontiguousarray(T.astype(dtype))


def _split_excess_waits(nc, limit=1):
    """This container's walrus rejects >1 sync wait per instruction (and any
    wait on Drain beyond its own barrier). Hoist extras onto same-engine
    NoOps placed immediately before."""
    import bass_rust
    import concourse.mybir as mybir

    n_split = 0
    for fn in nc.m.functions:
        for bb in fn.blocks:
            insts = bb.instructions
            i = 0
            while i < len(insts):
                inst = insts[i]
                si = inst.sync_info
                lim = 0 if type(inst).__name__ == "InstDrain" else limit
                if si is not None and si.on_wait and len(si.on_wait) > lim:
                    waits = list(si.on_wait)
                    keep, extra = waits[:lim], waits[lim:]
                    pos = i
                    for j in range(0, len(extra), max(limit, 1)):
                        ch = extra[j : j + max(limit, 1)]
                        nop = mybir.InstNoOp(
                            name=f"waitsplit_{n_split}_{pos}",
                            engine=inst.engine,
                            ins=[],
                            outs=[],
                            sync_info=bass_rust.SyncInfo(on_wait=ch, on_update=[]),
                        )
                        insts.insert(pos, nop)
                        pos += 1
                        n_split += 1
                    inst.sync_info = bass_rust.SyncInfo(
                        on_wait=keep, on_update=list(si.on_update)
                    )
                    i = pos + 1
                else:
                    i += 1
    return n_split


def _build_program():
    import concourse.bass as bass
    import concourse.mybir as mybir
    import concourse.tile as tile

    F32 = mybir.dt.float32
    F32R = mybir.dt.float32r
    BF16 = mybir.dt.bfloat16
    F8 = mybir.dt.float8e4
    AF = mybir.ActivationFunctionType
    OP = mybir.AluOpType

    nc = bass.Bass("TRN2", target_bir_lowering=False, debug=False, num_devices=8)

    # ---- DRAM parameters ----
    xin = nc.dram_tensor("x", [C, HW], BF16, kind="ExternalInput").ap()
    yin = nc.dram_tensor("y", [C, HW], BF16, kind="ExternalInput").ap()
    wq_d = nc.dram_tensor("wq", [C, C], BF16, kind="ExternalInput").ap()
    wkv_d = nc.dram_tensor("wkv", [C, 384], BF16, kind="ExternalInput").ap()
    wp_d = nc.dram_tensor("wp", [2, PC, C], BF16, kind="ExternalInput").ap()
    tq_d = nc.dram_tensor("tq", [128, C * 3, 128], F8, kind="ExternalInput").ap()
    tk_d = nc.dram_tensor("tk", [128, C * 7, 128], F8, kind="ExternalInput").ap()
    tv_d = nc.dram_tensor("tv", [128, C * 7, 128], BF16, kind="ExternalInput").ap()
    idb_d = nc.dram_tensor("idb", [128, 128], BF16, kind="ExternalInput").ap()
    mask_d = nc.dram_tensor("maskbd", [PC, PC], F32, kind="ExternalInput").ap()
    temp_d = nc.dram_tensor("temprow", [1, C], F32, kind="ExternalInput").ap()
    out_d = nc.dram_tensor("out", [C, HW], F32, kind="ExternalOutput").ap()

    with tile.TileContext(nc) as tc:
        import contextlib

        with contextlib.ExitStack() as ctx:
            consts = ctx.enter_context(tc.tile_pool(name="consts", bufs=1))
            s1 = ctx.enter_context(tc.tile_pool(name="s1", bufs=1))
            s2 = ctx.enter_context(tc.tile_pool(name="s2", bufs=1))
            s3 = ctx.enter_context(tc.tile_pool(name="s3", bufs=1))
            streams = ctx.enter_context(tc.tile_pool(name="streams", bufs=2))
            tpool8 = ctx.enter_context(tc.tile_pool(name="tpool8", bufs=5))
            tpoolb = ctx.enter_context(tc.tile_pool(name="tpoolb", bufs=3))
            ps = ctx.enter_context(tc.tile_pool(name="ps", bufs=4, space="PSUM"))
            pst = ctx.enter_context(tc.tile_pool(name="pst", bufs=2, space="PSUM"))
            scratch = ctx.enter_context(tc.tile_pool(name="scratch", bufs=2))
            ostage = ctx.enter_context(tc.tile_pool(name="ostage", bufs=2))
            misc = ctx.enter_context(tc.tile_pool(name="misc", bufs=4))
            attnp_pool = ctx.enter_context(tc.tile_pool(name="attnp", bufs=2, space="PSUM"))
            stats = ctx.enter_context(tc.tile_pool(name="stats", bufs=1))

            # ---- load constants ----
            wq0 = consts.tile([128, C], BF16)
            wq1 = consts.tile([64, C], BF16)
            nc.sync.dma_start(out=wq0, in_=wq_d[0:128, :])
            nc.sync.dma_start(out=wq1, in_=wq_d[128:192, :])
            wkv0 = consts.tile([128, 384], BF16)
            wkv1 = consts.tile([64, 384], BF16)
            nc.sync.dma_start(out=wkv0, in_=wkv_d[0:128, :])
            nc.sync.dma_start(out=wkv1, in_=wkv_d[128:192, :])
            wp0 = consts.tile([PC, C], BF16)
            wp1 = consts.tile([PC, C], BF16)
            nc.sync.dma_start(out=wp0, in_=wp_d[0])
            nc.sync.dma_start(out=wp1, in_=wp_d[1])
            identb = consts.tile([128, 128], BF16)
            nc.sync.dma_start(out=identb, in_=idb_d)
            ident8 = consts.tile([128, 128], F8)
            nc.vector.tensor_copy(ident8, identb)
            maskbd = consts.tile([PC, PC], F32)
            nc.sync.dma_start(out=maskbd, in_=mask_d)
            temprow = consts.tile([1, C], F32)
            nc.sync.dma_start(out=temprow, in_=temp_d)
            onescol = consts.tile([128, 1], BF16)
            nc.vector.memset(onescol, 1.0)
            ones1 = consts.tile([1, 128], BF16)
            nc.vector.memset(ones1, 1.0)

            # ---- big SBUF regions ----
            # channel-major: [w partitions, c*128 + h]; dw moving is contiguous
            bq = s1.tile([128, Himg * C], F8, tag="qv")
            bk = s2.tile([128, Himg * C + 32], F8, tag="kk")
            bv = s3.tile([128, Himg * C], BF16, tag="vv")
            bq3 = bq.rearrange("p (c h) -> p c h", h=Himg)
            bk3 = bk[:, 0 : Himg * C].rearrange("p (c h) -> p c h", h=Himg)
            bv3 = bv.rearrange("p (c h) -> p c h", h=Himg)
            # transposed views: [w, h, c] (strided in c)
            bq_hc = bq.rearrange("p (c h) -> p h c", h=Himg)
            bk_hc = bk[:, 0 : Himg * C].rearrange("p (c h) -> p h c", h=Himg)
            bv_hc = bv.rearrange("p (c h) -> p h c", h=Himg)

            # h-major staging ring for phase A (repacked to c-major by gpsimd)
            HSTG, CHUNK = 32, 16
            stg = stats.tile([128, HSTG * 384], BF16, name="stg")
            stg_hc = stg.rearrange("p (h c) -> p h c", c=384)
            stg_ch = stg.rearrange("p (h c) -> p c h", c=384)

            partials = stats.tile([128, 2 * C], F32)
            partials_bf = stats.tile([128, 2 * C], BF16)

            def chan_ap(region3, c, col0, cnt):
                # [128, cnt] contiguous view: channel c, h col0..col0+cnt
                return region3[:, c, col0 : col0 + cnt]

            def copy_on(eng_idx, dst, src):
                if eng_idx == 0:
                    nc.vector.tensor_copy(dst, src)
                else:
                    nc.scalar.activation(out=dst, in_=src, func=AF.Copy)

            # ================= Phase A: both 1x1 convs, interleaved =========
            # q uses the pst PSUM pool + a separate staging ring half; kv
            # uses ps + stg. Interleaving keeps the PE stream dense.
            rr = [0]
            HSTGQ = 16
            stgq = stats.tile([128, HSTGQ * C], BF16, name="stgq")
            stgq_hc = stgq.rearrange("p (h c) -> p h c", c=C)
            stgq_ch = stgq.rearrange("p (h c) -> p c h", c=C)

            def repack(rot, dst, src):
                e = rr[0] % rot
                rr[0] += 1
                if e == rot - 1:
                    nc.gpsimd.tensor_copy(dst, src)
                elif e % 2 == 0:
                    nc.vector.tensor_copy(dst, src)
                else:
                    nc.scalar.activation(out=dst, in_=src, func=AF.Copy)

            ptq, ptkv = [None], [None]
            for h in range(Himg):
                sl = h % SLAB
                if sl == 0:
                    qs0 = streams.tile([128, SLAB * 128], BF16, tag="qs0")
                    qs1 = streams.tile([64, SLAB * 128], BF16, tag="qs1")
                    nc.sync.dma_start(out=qs0, in_=xin[0:128, h * 128 : (h + SLAB) * 128])
                    nc.sync.dma_start(out=qs1, in_=xin[128:192, h * 128 : (h + SLAB) * 128])
                    ys0 = streams.tile([128, SLAB * 128], BF16, tag="ys0")
                    ys1 = streams.tile([64, SLAB * 128], BF16, tag="ys1")
                    nc.sync.dma_start(out=ys0, in_=yin[0:128, h * 128 : (h + SLAB) * 128])
                    nc.sync.dma_start(out=ys1, in_=yin[128:192, h * 128 : (h + SLAB) * 128])
                # ---- q conv (h2=2, pst pool) ----
                if h % 2 == 0:
                    ptq[0] = pst.tile([128, 2 * C], F32, tag="tp", name=f"ptq_{h}")
                offq = (h % 2) * C
                nc.tensor.matmul(
                    ptq[0][:, offq : offq + C],
                    qs0[:, sl * 128 : (sl + 1) * 128], wq0,
                    start=True, stop=False,
                )
                nc.tensor.matmul(
                    ptq[0][:, offq : offq + C],
                    qs1[:, sl * 128 : (sl + 1) * 128], wq1,
                    start=False, stop=True,
                )
                # ---- kv conv (h2=1, ps pool) ----
                ptkv[0] = ps.tile([128, 384], F32, tag="ps", name=f"ptkv_{h}")
                nc.tensor.matmul(
                    ptkv[0], ys0[:, sl * 128 : (sl + 1) * 128], wkv0,
                    start=True, stop=False,
                )
                nc.tensor.matmul(
                    ptkv[0], ys1[:, sl * 128 : (sl + 1) * 128], wkv1,
                    start=False, stop=True,
                )
                # ---- writebacks to staging ----
                if h % 2 == 1:
                    copy_on((h // 2) % 2, stgq_hc[:, (h - 1) % HSTGQ : (h - 1) % HSTGQ + 2, :], ptq[0])
                copy_on(h % 2, stg_hc[:, h % HSTG, :], ptkv[0])
                # ---- chunk repacks ----
                if h % 8 == 7:
                    hq0 = h - 7
                    sq0 = hq0 % HSTGQ
                    repack(6, bq3[:, 0:96, hq0 : hq0 + 8], stgq_ch[:, 0:96, sq0 : sq0 + 8])
                    repack(6, bq3[:, 96:192, hq0 : hq0 + 8], stgq_ch[:, 96:192, sq0 : sq0 + 8])
                if h % CHUNK == CHUNK - 1:
                    hc0 = h - CHUNK + 1
                    s0 = hc0 % HSTG
                    for reg3, c0, c1 in ((bk3, 0, C), (bv3, C, 2 * C)):
                        half = (c0 + c1) // 2
                        for cl, cr in ((c0, half), (half, c1)):
                            repack(6, reg3[:, cl - c0 : cr - c0, hc0 : hc0 + CHUNK],
                                   stg_ch[:, cl:cr, s0 : s0 + CHUNK])

            # ================= Phase B: depthwise via Toeplitz matmuls ======
            GB = 4                      # channels per PSUM bank group

            def dw_phase(region3, t_dram, ksz, tdt, tag, sq_off=None):
                pad = ksz // 2
                order = [pad] + [d for d in range(ksz) if d != pad]
                cw = 4                      # channels per T-wave
                wave_tile = [None]
                pdw4 = [None]
                for ci in range(C):
                    if ci % cw == 0:
                        nt = min(cw, C - ci) * ksz
                        wave_tile[0] = (tpool8 if tag == "tw8" else tpoolb).tile([128, cw * ksz, 128], tdt, tag=tag, name=f"tw_{tag}_{ci}")
                        i0 = ci * ksz
                        nc.sync.dma_start(
                            out=wave_tile[0][:, 0:nt, :],
                            in_=t_dram[:, i0 : i0 + nt, :],
                        )
                    tw = wave_tile[0]
                    if ci % GB == 0:
                        pdw4[0] = ps.tile([128, GB * 128], F32, tag="ps", name=f"pdw_{tag}_{ci}")
                    base = (ci % cw) * ksz
                    slot = (ci % GB) * 128
                    for j, dh in enumerate(order):
                        sh = dh - pad
                        cnt = Himg - abs(sh)
                        h0o, h0i = max(0, -sh), max(0, sh)
                        nc.tensor.matmul(
                            pdw4[0][:, slot + h0o : slot + h0o + cnt],
                            tw[:, base + dh, :],
                            chan_ap(region3, ci, h0i, cnt),
                            start=(j == 0),
                            stop=(j == len(order) - 1),
                        )
                    if ci % GB == GB - 1:
                        g0 = ci - (GB - 1)
                        # group writeback: DVE for q/k (ACT busy with squares),
                        # alternate for v
                        eng = (ci // GB) % 2 if sq_off is None else 0
                        copy_on(eng, region3[:, g0 : ci + 1, :], pdw4[0])
                        if sq_off is not None:
                            # sum-of-squares: batched Square on ACT, then
                            # per-channel reduce on DVE
                            sc = scratch.tile([128, GB * 128], BF16, tag="sq", name=f"sq_{tag}_{ci}")
                            nc.scalar.activation(
                                out=sc,
                                in_=region3[:, g0 : ci + 1, :],
                                func=AF.Square,
                            )
                            sc3 = sc.rearrange("p (c h) -> p c h", h=Himg)
                            nc.vector.tensor_reduce(
                                partials[:, sq_off + g0 : sq_off + ci + 1],
                                sc3,
                                axis=mybir.AxisListType.X,
                                op=OP.add,
                            )

            dw_phase(bq3, tq_d, 3, F8, "tw8", sq_off=0)
            dw_phase(bk3, tk_d, 7, F8, "tw8", sq_off=C)
            nc.vector.tensor_copy(partials_bf, partials)

            # ================= Phase D: QK^T + softmax prep per pair ========
            ezs = []
            for P in range(2):
                attnp = attnp_pool.tile([PC, PC], F32, tag="at")
                for h in range(Himg):
                    nc.tensor.matmul(
                        attnp,
                        bk_hc[:, h, PC * P : PC * P + PC],
                        bq_hc[:, h, PC * P : PC * P + PC],
                        start=(h == 0),
                        stop=(h == Himg - 1),
                    )
                # rq as a row [1, PC]: colsum of q-partials then rsqrt, * temp
                prow = ps.tile([1, PC], F32, tag="ps")
                nc.tensor.matmul(
                    prow, onescol, partials_bf[:, PC * P : PC * P + PC],
                    start=True, stop=True,
                )
                sq_row = misc.tile([1, PC], F32, tag="m1")
                nc.scalar.activation(out=sq_row, in_=prow, func=AF.Sqrt)
                rq_row = misc.tile([1, PC], F32, tag="m2")
                nc.vector.reciprocal(rq_row, sq_row)
                nc.vector.tensor_tensor(
                    rq_row, rq_row, temprow[:, PC * P : PC * P + PC], op=OP.mult
                )
                rq_bf = misc.tile([1, PC], BF16, tag="m3")
                nc.vector.tensor_copy(rq_bf, rq_row)
                # rk as a column [PC, 1]
                pcol = ps.tile([PC, 1], F32, tag="ps")
                nc.tensor.matmul(
                    pcol, partials_bf[:, C + PC * P : C + PC * P + PC], onescol,
                    start=True, stop=True,
                )
                sq_col = misc.tile([PC, 1], F32, tag="m4")
                nc.scalar.activation(out=sq_col, in_=pcol, func=AF.Sqrt)
                rk_col = misc.tile([PC, 1], F32, tag="m5")
                nc.vector.reciprocal(rk_col, sq_col)
                # rq replicated across partitions via K=1 matmul
                prep = ps.tile([PC, PC], F32, tag="ps")
                nc.tensor.matmul(
                    prep, ones1[:, 0:PC], rq_bf, start=True, stop=True
                )
                rqrep = misc.tile([PC, PC], F32, tag="m6")
                nc.vector.tensor_copy(rqrep, prep)
                t1 = misc.tile([PC, PC], F32, tag="m7")
                nc.vector.tensor_tensor(t1, attnp, rqrep, op=OP.mult)
                # exp(rk * t1), then zero junk blocks, bf16
                e1 = misc.tile([PC, PC], F32, tag="m8")
                nc.scalar.activation(out=e1, in_=t1, func=AF.Exp, scale=rk_col)
                ezero = stats.tile([PC, 128], BF16, tag=f"ez{P}")
                nc.vector.memset(ezero[:, PC:128], 0.0)
                nc.vector.tensor_tensor(ezero[:, 0:PC], e1, maskbd, op=OP.mult)
                # column sums -> recip
                pcs = ps.tile([PC, 1], F32, tag="ps")
                nc.tensor.matmul(
                    pcs, ezero[:, 0:PC], onescol[0:PC], start=True, stop=True
                )
                recip = stats.tile([PC, 1], F32, tag=f"rc{P}")
                nc.vector.reciprocal(recip, pcs)
                ezs.append((ezero, recip))

            # ================= Phase E: v depthwise =========================
            dw_phase(bv3, tv_d, 7, BF16, "twb")

            # ================= Phase G: fused (attn @ v) + proj =============
            mps = []
            for P in range(2):
                ezero, recip = ezs[P]
                ezt_ps = pst.tile([PC, PC], BF16, tag="tp")
                nc.tensor.transpose(ezt_ps, ezero[:, 0:PC], identb[0:PC, 0:PC])
                ezt = misc.tile([PC, PC], BF16, tag="m9")
                nc.vector.tensor_copy(ezt, ezt_ps)
                wsc = misc.tile([PC, C], BF16, tag="m10")
                nc.vector.tensor_scalar_mul(wsc, (wp0, wp1)[P], recip)
                pmp = ps.tile([PC, C], F32, tag="ps")
                nc.tensor.matmul(pmp, ezt, wsc, start=True, stop=True)
                mp = stats.tile([PC, C], BF16, tag=f"mp{P}")
                nc.vector.tensor_copy(mp, pmp)
                mps.append(mp)

            # per 512-col block: transpose 4 h-rows of v per pair (batched
            # into one PSUM tile), then the two fused output matmuls
            for nb in range(Himg // 4):
                h0 = nb * 4
                vtbs = []
                for P in range(2):
                    ptv = pst.tile([PC, 512], BF16, tag="tp", name=f"ptv{P}_{nb}")
                    for hh in range(4):
                        nc.tensor.transpose(
                            ptv[:, hh * 128 : (hh + 1) * 128],
                            bv_hc[:, h0 + hh, PC * P : PC * P + PC],
                            identb,
                        )
                    vtb = scratch.tile([PC, 512], BF16, tag=f"vtb{P}", name=f"vtb{P}_{nb}")
                    copy_on(P, vtb, ptv)
                    vtbs.append(vtb)
                n = nb * 512
                for mi, (r0, r1) in enumerate(((0, 128), (128, 192))):
                    mw = r1 - r0
                    po = ps.tile([mw, 512], F32, tag="ps", name=f"po_{mi}_{nb}")
                    nc.tensor.matmul(
                        po, mps[0][:, r0:r1], vtbs[0],
                        start=True, stop=False,
                    )
                    nc.tensor.matmul(
                        po, mps[1][:, r0:r1], vtbs[1],
                        start=False, stop=True,
                    )
                    so = ostage.tile([mw, 512], F32, tag="os", name=f"so_{mi}_{nb}")
                    copy_on(mi, so, po)
                    nc.sync.dma_start(out=out_d[r0:r1, n : n + 512], in_=so)

    _split_excess_waits(nc)
    return nc


def _get_program():
    global _PROG
    if _PROG is None:
        _PROG = _build_program()
    return _PROG


def kernel(x, y, q_w, q_dw_w, kv_w, kv_dw_w, proj_w, temperature):
    return _run(x, y, q_w, q_dw_w, kv_w, kv_dw_w, proj_w, temperature)[0]


def _run(x, y, q_w, q_dw_w, kv_w, kv_dw_w, proj_w, temperature, trace=False):
    from concourse.bass_utils import run_bass_kernel_spmd

    x = np.asarray(x, dtype=np.float32).reshape(B, C, HW).astype(ml_dtypes.bfloat16)
    y = np.asarray(y, dtype=np.float32).reshape(B, C, HW).astype(ml_dtypes.bfloat16)
    q_w = np.asarray(q_w, dtype=np.float32)
    kv_w = np.asarray(kv_w, dtype=np.float32)
    proj_w = np.asarray(proj_w, dtype=np.float32)
    q_dw_w = np.asarray(q_dw_w, dtype=np.float32)
    kv_dw_w = np.asarray(kv_dw_w, dtype=np.float32)
    temperature = np.asarray(temperature, dtype=np.float32).reshape(HEADS)

    wq = np.ascontiguousarray(q_w[:, :, 0, 0].T.astype(ml_dtypes.bfloat16))
    wkv = np.ascontiguousarray(kv_w[:, :, 0, 0].T.astype(ml_dtypes.bfloat16))  # [C, 2C]
    wpT = proj_w[:, :, 0, 0].T                              # [c_in, c_out]
    wp = np.stack([wpT[0:PC], wpT[PC:C]]).astype(ml_dtypes.bfloat16)
    tq = _build_toeplitz(q_dw_w[:, 0], 3, ml_dtypes.float8_e4m3)
    tk = _build_toeplitz(kv_dw_w[0:C, 0], 7, ml_dtypes.float8_e4m3)
    tv = _build_toeplitz(kv_dw_w[C : 2 * C, 0], 7, ml_dtypes.bfloat16)
    idb = np.eye(128, dtype=ml_dtypes.bfloat16)
    maskbd = np.zeros((PC, PC), np.float32)
    maskbd[0:DHC, 0:DHC] = 1.0
    maskbd[DHC:PC, DHC:PC] = 1.0
    temprow = np.repeat(temperature, DHC).reshape(1, C)

    shared = {
        "wq": wq, "wkv": wkv, "wp": wp, "tq": tq, "tk": tk, "tv": tv,
        "idb": idb, "maskbd": maskbd, "temprow": temprow,
    }
    in_maps = [dict(shared, x=x[i], y=y[i]) for i in range(B)]

    nc = _get_program()
    res = run_bass_kernel_spmd(
        nc, in_maps, core_ids=list(range(B)), trace=trace
    )
    out = np.stack([res.results[i]["out"] for i in range(B)])
    return out.reshape(B, C, Himg, Wimg).astype(np.float32), res

